# revision 7
# baseline (speedup 1.0000x reference)
"""Trainium2 Bass kernel for BinaryConv2dLayer — fp8 DoubleRow, e3m4 output.

Reference op: W_b = sign(W) * (sum(W)/sum(sgn(W))); y = relu(conv2d_SAME(x, W_b)).
x: [16, 256, 256, 64] NHWC fp32, W: [3, 3, 64, 64] HWIO fp32.

Data-parallel: 2 images per core on 8 cores. Per core:
- Host: weights binarized to exact +-1 in fp8e4m3; x = hi + lo with
  hi = e4m3(x), lo = e4m3(x - hi) (input path err ~8e-4).
- Layout ("O-pairs"): partitions = (row-slot, 64ch); O-pair q holds rows
  (2q-1, 2q) [odd row on parts 0:64, even on 64:128], q = 0..128 per image,
  rows padded to 260 cols (zeros at 0 and 257..259). An output row-pair
  (2p, 2p+1) takes all 18 conv taps from O-pairs p and p+1 only => input
  loaded once, no cross-shifted copy.
- Per 512-col PSUM block: 6 chained fp8 DoubleRow matmuls (3 hi + 3 lo; one
  per dx), each contracting K-tiles A=O-pair p and B=O-pair p+1 at dim-1
  stride 260 (0.5 cyc/row = 4x bf16 MAC rate). NOTE: the hw DR ifmap
  fetcher crashes for some dim-1 strides (1/256/257 observed); 260 is
  hw-validated, which is why rows are padded to 260 not 258.
- Epilogue relu(8*scale*psum) -> float8e3 (e3m4) alternating DVE/ACT;
  output stored as e3m4 (1B) halving store traffic; host divides by 8.
  e3m4 output quantization dominates the error: ~1.3e-2 rel (gate 2e-2).
- Loads issue on SP, stores on ACT; DMA ~72us busy, PE ~83us -> PE-bound.
"""

import numpy as np
import ml_dtypes

F8 = ml_dtypes.float8_e4m3
E3 = ml_dtypes.float8_e3m4

H = 256
WD = 256
C = 64
COLW = 258                 # padded row width (= DR pair stride, probed on hw)
NPAIR = 129                # O-pairs per image
PAD = 4
FL = 2 * PAD + NPAIR * COLW
GRID = 128 * COLW          # out grid cols per image (33024)
NIMG = 16
NCORES = 8
IPC = NIMG // NCORES
NBLK = 512
OSLAB = 3 * NBLK           # out cols per store chunk
PSUM_BUFS = 8
OUT_BUFS = 16
PRE8 = 8.0                 # output prescale (uses e3m4 range; host undoes)
# per-image input chunk boundaries in O-pair index (chunk0 small: fast start)
CHUNKS_FIRST = (0, 3, 10, 20, 36, 52, 68, 84, 100, 116, 129)
CHUNKS_REST = (0, 43, 86, 129)

_PROG = {}


def _pv(t, off, n, stride):
    v = t[:, off:off + n].unsqueeze(1)
    v.ap[1] = [stride, 2]
    return v


def _build_program(scale):
    import concourse.mybir as mybir
    from concourse import bacc
    from concourse.tile import TileContext

    dt = mybir.dt
    nc = bacc.Bacc("TRN2")
    xhi = nc.dram_tensor("xhi", [128, IPC * FL], dt.float8e4, kind="ExternalInput")
    xlo = nc.dram_tensor("xlo", [128, IPC * FL], dt.float8e4, kind="ExternalInput")
    wdr = nc.dram_tensor("wdr", [128, 3 * 256], dt.float8e4, kind="ExternalInput")
    y = nc.dram_tensor("y", [128, IPC * GRID], dt.float8e3, kind="ExternalOutput")
    relu = mybir.ActivationFunctionType.Relu
    s8 = float(PRE8 * scale)

    with TileContext(nc) as tc:
        with (
            tc.tile_pool(name="wpool", bufs=1) as wpool,
            tc.tile_pool(name="inp", bufs=2) as inp,
            tc.tile_pool(name="psum", bufs=PSUM_BUFS, space="PSUM") as psump,
            tc.tile_pool(name="outp", bufs=OUT_BUFS) as outp,
        ):
            wt = wpool.tile([128, 3 * 256], dt.float8e4)
            nc.sync.dma_start(out=wt[:], in_=wdr[:])

            # Front-load all input DMAs (both images fit in SBUF): transfers
            # queue ahead of stores on the DMA device so PE is never starved.
            tiles = []
            for j in range(IPC):
                hi = inp.tile([128, FL], dt.float8e4, tag="hi")
                lo = inp.tile([128, FL], dt.float8e4, tag="lo")
                tiles.append((hi, lo))
                bounds = CHUNKS_FIRST if j == 0 else CHUNKS_REST
                for ci in range(len(bounds) - 1):
                    c0 = 0 if ci == 0 else PAD + bounds[ci] * COLW
                    c1 = FL if ci == len(bounds) - 2 else PAD + bounds[ci + 1] * COLW
                    src0 = j * FL + c0
                    nc.sync.dma_start(out=hi[:, c0:c1], in_=xhi[:, src0:src0 + (c1 - c0)])
                    nc.sync.dma_start(out=lo[:, c0:c1], in_=xlo[:, src0:src0 + (c1 - c0)])

            eidx = 0
            for j in range(IPC):
                hi, lo = tiles[j]
                slabs = []
                S = 0
                while S < GRID:
                    SL = min(OSLAB, GRID - S)
                    slabs.append((S, SL))
                    S += SL
                for si, (S, SL) in enumerate(slabs):
                    last = (j == IPC - 1 and si == len(slabs) - 1)
                    ot = outp.tile([128, OSLAB], dt.float8e3, tag="ot")
                    for T in range(S, S + SL, NBLK):
                        N = min(NBLK, S + SL - T)
                        ps = psump.tile([128, NBLK], dt.float32, tag="ps")
                        a = PAD + T
                        for comp_i, comp in enumerate((hi, lo)):
                            for dxi in range(3):
                                nc.tensor.matmul(
                                    ps[:, :N],
                                    _pv(wt, dxi * 256, 128, 128),
                                    _pv(comp, a + dxi - 1, N, COLW),
                                    start=(comp_i == 0 and dxi == 0),
                                    stop=(comp_i == 1 and dxi == 2),
                                    perf_mode=mybir.MatmulPerfMode.DoubleRow,
                                )
                        if eidx % 2 == 0 or last:
                            nc.vector.tensor_scalar(
                                out=ot[:, T - S:T - S + N], in0=ps[:, :N],
                                scalar1=s8, scalar2=0.0,
                                op0=mybir.AluOpType.mult,
                                op1=mybir.AluOpType.max)
                        else:
                            nc.scalar.activation(
                                out=ot[:, T - S:T - S + N], in_=ps[:, :N],
                                func=relu, scale=s8)
                        eidx += 1
                    dst = j * GRID + S
                    eng = nc.sync if last else nc.scalar
                    eng.dma_start(out=y[:, dst:dst + SL], in_=ot[:, :SL])
    nc.finalize()
    return nc


def _get_program(scale):
    key = float(scale)
    if key not in _PROG:
        _PROG[key] = _build_program(key)
    return _PROG[key]


def _host_prep_x(x):
    xf = np.ascontiguousarray(x, dtype=np.float32)
    hi = xf.astype(F8)
    lo = (xf - hi.astype(np.float32)).astype(F8)
    out = []
    for comp in (hi, lo):
        arr = comp.reshape(NCORES, IPC, H, WD, C)
        flat = np.zeros((NCORES, 128, IPC * FL), dtype=F8)
        for j in range(IPC):
            base = j * FL + PAD
            view = flat[:, :, base:base + NPAIR * COLW].reshape(NCORES, 128, NPAIR, COLW)
            # odd rows 1,3,..,255 -> q=1..128, partitions 0:64
            view[:, 0:64, 1:129, 1:257] = arr[:, j, 1::2].transpose(0, 3, 1, 2)
            # even rows 0,2,..,254 -> q=0..127, partitions 64:128
            view[:, 64:128, 0:128, 1:257] = arr[:, j, 0::2].transpose(0, 3, 1, 2)
        out.append(flat)
    return out


def _host_prep_w(W):
    Wf = np.ascontiguousarray(W).astype(np.float32)
    sgn = np.sign(Wf)
    scale = np.float32(Wf.sum(dtype=np.float32) / sgn.sum(dtype=np.float32))
    s8 = sgn.astype(F8)  # exact +-1 (0 for exact-zero W entries)
    # K: 0:64 odd row (2q-1), 64:128 even row (2q)
    # M: 0:64 out even row 2p, 64:128 out odd row 2p+1
    WA = np.zeros((3, 128, 128), dtype=F8)
    WB = np.zeros((3, 128, 128), dtype=F8)
    for kx in range(3):
        WA[kx, 0:64, 0:64] = s8[0, kx]      # row 2p-1 -> out 2p   (ky=0)
        WA[kx, 64:128, 0:64] = s8[1, kx]    # row 2p   -> out 2p   (ky=1)
        WA[kx, 64:128, 64:128] = s8[0, kx]  # row 2p   -> out 2p+1 (ky=0)
        WB[kx, 0:64, 0:64] = s8[2, kx]      # row 2p+1 -> out 2p   (ky=2)
        WB[kx, 0:64, 64:128] = s8[1, kx]    # row 2p+1 -> out 2p+1 (ky=1)
        WB[kx, 64:128, 64:128] = s8[2, kx]  # row 2p+2 -> out 2p+1 (ky=2)
    # DR matmul dxi pairs K-tile A (O-pair p) with K-tile B (O-pair p+1)
    wdr = np.zeros((128, 3 * 256), dtype=F8)
    for dxi in range(3):
        wdr[:, dxi * 256:dxi * 256 + 128] = WA[dxi]
        wdr[:, dxi * 256 + 128:dxi * 256 + 256] = WB[dxi]
    return wdr, scale


def _unshard(results):
    inv = np.float32(1.0 / PRE8)
    out = np.empty((NIMG, H, WD, C), dtype=np.float32)
    for k in range(NCORES):
        yk = results[k]["y"]
        for j in range(IPC):
            o = yk[:, j * GRID:(j + 1) * GRID].astype(np.float32) * inv
            o = o.reshape(2, 64, 128, COLW)[:, :, :, 1:257]
            out[k * IPC + j] = o.transpose(2, 0, 3, 1).reshape(H, WD, C)
    return out


def kernel(x, W):
    from concourse.bass_utils import run_bass_kernel_spmd

    hi, lo = _host_prep_x(np.asarray(x))
    wdr, scale = _host_prep_w(np.asarray(W))
    nc = _get_program(scale)
    in_maps = [
        {"xhi": np.ascontiguousarray(hi[k]), "xlo": np.ascontiguousarray(lo[k]),
         "wdr": wdr}
        for k in range(NCORES)
    ]
    res = run_bass_kernel_spmd(nc, in_maps, core_ids=list(range(NCORES)))
    return _unshard(res.results)


# revision 8
# speedup vs baseline: 1.0017x; 1.0017x over previous
"""Trainium2 Bass kernel for BinaryConv2dLayer — fp8 DoubleRow, e3m4 output.

Reference op: W_b = sign(W) * (sum(W)/sum(sgn(W))); y = relu(conv2d_SAME(x, W_b)).
x: [16, 256, 256, 64] NHWC fp32, W: [3, 3, 64, 64] HWIO fp32.

Data-parallel: 2 images per core on 8 cores. Per core:
- Host: weights binarized to exact +-1 in fp8e4m3; x = hi + lo with
  hi = e4m3(x), lo = e4m3(x - hi) (input path err ~8e-4).
- Layout ("O-pairs"): partitions = (row-slot, 64ch); O-pair q holds rows
  (2q-1, 2q) [odd row on parts 0:64, even on 64:128], q = 0..128 per image,
  rows padded to 258 cols (zero col at 0 and 257). An output row-pair
  (2p, 2p+1) takes all 18 conv taps from O-pairs p and p+1 only => input
  loaded once, no cross-shifted copy.
- Per 512-col PSUM block: 6 chained fp8 DoubleRow matmuls (3 hi + 3 lo; one
  per dx), each contracting K-tiles A=O-pair p and B=O-pair p+1 at dim-1
  stride 258 (0.5 cyc/row = 4x bf16 MAC rate). NOTE: the hw DR ifmap
  fetcher crashes the exec unit for dim-1 strides 1, 256, 257 (probed);
  2, 8, 128, 258, 260, 300, 384, 512, 516, 700, 16770, 16772 pass.
- Epilogue relu(8*scale*psum) -> float8e3 (e3m4) alternating DVE/ACT;
  output stored as e3m4 (1B) halving store traffic; host divides by 8.
  e3m4 output quantization dominates the error: ~1.3e-2 rel (gate 2e-2).
- Loads issue on SP, stores on ACT; the final 768-col tail slab runs its
  epilogues on DVE and stores on the otherwise-idle SP queue to shorten the
  end-of-program chain. DMA ~71us busy, PE ~83us -> PE-bound.
Modeled + hw-verified: 91848 ns/core, rel err 1.33e-2 (vs 174780 ns
baseline: 1.90x).
"""

import numpy as np
import ml_dtypes

F8 = ml_dtypes.float8_e4m3
E3 = ml_dtypes.float8_e3m4

H = 256
WD = 256
C = 64
COLW = 258                 # padded row width (= DR pair stride, probed on hw)
NPAIR = 129                # O-pairs per image
PAD = 4
FL = 2 * PAD + NPAIR * COLW
GRID = 128 * COLW          # out grid cols per image (33024)
NIMG = 16
NCORES = 8
IPC = NIMG // NCORES
NBLK = 512
OSLAB = 3 * NBLK           # out cols per store chunk
PSUM_BUFS = 8
OUT_BUFS = 16
PRE8 = 8.0                 # output prescale (uses e3m4 range; host undoes)
# per-image input chunk boundaries in O-pair index (chunk0 small: fast start)
CHUNKS_FIRST = (0, 3, 10, 20, 36, 52, 68, 84, 100, 116, 129)
CHUNKS_REST = (0, 43, 86, 129)

_PROG = {}


def _pv(t, off, n, stride):
    v = t[:, off:off + n].unsqueeze(1)
    v.ap[1] = [stride, 2]
    return v


def _build_program(scale):
    import concourse.mybir as mybir
    from concourse import bacc
    from concourse.tile import TileContext

    dt = mybir.dt
    nc = bacc.Bacc("TRN2")
    xhi = nc.dram_tensor("xhi", [128, IPC * FL], dt.float8e4, kind="ExternalInput")
    xlo = nc.dram_tensor("xlo", [128, IPC * FL], dt.float8e4, kind="ExternalInput")
    wdr = nc.dram_tensor("wdr", [128, 3 * 256], dt.float8e4, kind="ExternalInput")
    y = nc.dram_tensor("y", [128, IPC * GRID], dt.float8e3, kind="ExternalOutput")
    relu = mybir.ActivationFunctionType.Relu
    s8 = float(PRE8 * scale)

    with TileContext(nc) as tc:
        with (
            tc.tile_pool(name="wpool", bufs=1) as wpool,
            tc.tile_pool(name="inp", bufs=2) as inp,
            tc.tile_pool(name="psum", bufs=PSUM_BUFS, space="PSUM") as psump,
            tc.tile_pool(name="outp", bufs=OUT_BUFS) as outp,
        ):
            wt = wpool.tile([128, 3 * 256], dt.float8e4)
            nc.sync.dma_start(out=wt[:], in_=wdr[:])

            # Front-load all input DMAs (both images fit in SBUF): transfers
            # queue ahead of stores on the DMA device so PE is never starved.
            tiles = []
            for j in range(IPC):
                hi = inp.tile([128, FL], dt.float8e4, tag="hi")
                lo = inp.tile([128, FL], dt.float8e4, tag="lo")
                tiles.append((hi, lo))
                bounds = CHUNKS_FIRST if j == 0 else CHUNKS_REST
                for ci in range(len(bounds) - 1):
                    c0 = 0 if ci == 0 else PAD + bounds[ci] * COLW
                    c1 = FL if ci == len(bounds) - 2 else PAD + bounds[ci + 1] * COLW
                    src0 = j * FL + c0
                    nc.sync.dma_start(out=hi[:, c0:c1], in_=xhi[:, src0:src0 + (c1 - c0)])
                    nc.sync.dma_start(out=lo[:, c0:c1], in_=xlo[:, src0:src0 + (c1 - c0)])

            eidx = 0
            for j in range(IPC):
                hi, lo = tiles[j]
                slabs = []
                S = 0
                while S < GRID:
                    SL = min(OSLAB, GRID - S)
                    slabs.append((S, SL))
                    S += SL
                for si, (S, SL) in enumerate(slabs):
                    last = (j == IPC - 1 and si == len(slabs) - 1)
                    ot = outp.tile([128, OSLAB], dt.float8e3, tag="ot")
                    for T in range(S, S + SL, NBLK):
                        N = min(NBLK, S + SL - T)
                        ps = psump.tile([128, NBLK], dt.float32, tag="ps")
                        a = PAD + T
                        for comp_i, comp in enumerate((hi, lo)):
                            for dxi in range(3):
                                nc.tensor.matmul(
                                    ps[:, :N],
                                    _pv(wt, dxi * 256, 128, 128),
                                    _pv(comp, a + dxi - 1, N, COLW),
                                    start=(comp_i == 0 and dxi == 0),
                                    stop=(comp_i == 1 and dxi == 2),
                                    perf_mode=mybir.MatmulPerfMode.DoubleRow,
                                )
                        if eidx % 2 == 0 or last:
                            nc.vector.tensor_scalar(
                                out=ot[:, T - S:T - S + N], in0=ps[:, :N],
                                scalar1=s8, scalar2=0.0,
                                op0=mybir.AluOpType.mult,
                                op1=mybir.AluOpType.max)
                        else:
                            nc.scalar.activation(
                                out=ot[:, T - S:T - S + N], in_=ps[:, :N],
                                func=relu, scale=s8)
                        eidx += 1
                    dst = j * GRID + S
                    eng = nc.sync if last else nc.scalar
                    eng.dma_start(out=y[:, dst:dst + SL], in_=ot[:, :SL])
    nc.finalize()
    return nc


def _get_program(scale):
    key = float(scale)
    if key not in _PROG:
        _PROG[key] = _build_program(key)
    return _PROG[key]


def _host_prep_x(x):
    xf = np.ascontiguousarray(x, dtype=np.float32)
    hi = xf.astype(F8)
    lo = (xf - hi.astype(np.float32)).astype(F8)
    out = []
    for comp in (hi, lo):
        arr = comp.reshape(NCORES, IPC, H, WD, C)
        flat = np.zeros((NCORES, 128, IPC * FL), dtype=F8)
        for j in range(IPC):
            base = j * FL + PAD
            view = flat[:, :, base:base + NPAIR * COLW].reshape(NCORES, 128, NPAIR, COLW)
            # odd rows 1,3,..,255 -> q=1..128, partitions 0:64
            view[:, 0:64, 1:129, 1:257] = arr[:, j, 1::2].transpose(0, 3, 1, 2)
            # even rows 0,2,..,254 -> q=0..127, partitions 64:128
            view[:, 64:128, 0:128, 1:257] = arr[:, j, 0::2].transpose(0, 3, 1, 2)
        out.append(flat)
    return out


def _host_prep_w(W):
    Wf = np.ascontiguousarray(W).astype(np.float32)
    sgn = np.sign(Wf)
    scale = np.float32(Wf.sum(dtype=np.float32) / sgn.sum(dtype=np.float32))
    s8 = sgn.astype(F8)  # exact +-1 (0 for exact-zero W entries)
    # K: 0:64 odd row (2q-1), 64:128 even row (2q)
    # M: 0:64 out even row 2p, 64:128 out odd row 2p+1
    WA = np.zeros((3, 128, 128), dtype=F8)
    WB = np.zeros((3, 128, 128), dtype=F8)
    for kx in range(3):
        WA[kx, 0:64, 0:64] = s8[0, kx]      # row 2p-1 -> out 2p   (ky=0)
        WA[kx, 64:128, 0:64] = s8[1, kx]    # row 2p   -> out 2p   (ky=1)
        WA[kx, 64:128, 64:128] = s8[0, kx]  # row 2p   -> out 2p+1 (ky=0)
        WB[kx, 0:64, 0:64] = s8[2, kx]      # row 2p+1 -> out 2p   (ky=2)
        WB[kx, 0:64, 64:128] = s8[1, kx]    # row 2p+1 -> out 2p+1 (ky=1)
        WB[kx, 64:128, 64:128] = s8[2, kx]  # row 2p+2 -> out 2p+1 (ky=2)
    # DR matmul dxi pairs K-tile A (O-pair p) with K-tile B (O-pair p+1)
    wdr = np.zeros((128, 3 * 256), dtype=F8)
    for dxi in range(3):
        wdr[:, dxi * 256:dxi * 256 + 128] = WA[dxi]
        wdr[:, dxi * 256 + 128:dxi * 256 + 256] = WB[dxi]
    return wdr, scale


def _unshard(results):
    inv = np.float32(1.0 / PRE8)
    out = np.empty((NIMG, H, WD, C), dtype=np.float32)
    for k in range(NCORES):
        yk = results[k]["y"]
        for j in range(IPC):
            o = yk[:, j * GRID:(j + 1) * GRID].astype(np.float32) * inv
            o = o.reshape(2, 64, 128, COLW)[:, :, :, 1:257]
            out[k * IPC + j] = o.transpose(2, 0, 3, 1).reshape(H, WD, C)
    return out


def kernel(x, W):
    from concourse.bass_utils import run_bass_kernel_spmd

    hi, lo = _host_prep_x(np.asarray(x))
    wdr, scale = _host_prep_w(np.asarray(W))
    nc = _get_program(scale)
    in_maps = [
        {"xhi": np.ascontiguousarray(hi[k]), "xlo": np.ascontiguousarray(lo[k]),
         "wdr": wdr}
        for k in range(NCORES)
    ]
    res = run_bass_kernel_spmd(nc, in_maps, core_ids=list(range(NCORES)))
    return _unshard(res.results)
